# revision 1
# baseline (speedup 1.0000x reference)
"""Trainium2 Bass kernel v3 for nn_BinLinear: out = x @ where(w >= 0, 1, -1).

Variants (BL3_VARIANT env or get_nc(variant)):
  fp8dr9 - fp8 DoubleRow hi over all K + fp8 lo over 2304 k.  err ~1.756e-2
  fp8dr8 - same, lo over 2048 k.                              err ~1.876e-2
  hyb16  - fp8 DR hi-only over k<2048, fp16 normal matmul
           (mixed dtype vs resident fp8 w) over k>=2048.      err ~1.877e-2

All: x row-sharded 8 cores (1024 rows), w resident in SBUF as fp8 e4m3
(+-1 exact), x-stationary (8 moving w panels per LDWEIGHTS), out natural
[1024, 4096] fp32/core, host concat.  DoubleRow pair k-map:
k = kc*256 + i*128 + p.
"""

import os
import sys

for _p in ("/opt/trn_rl_repo", "/root/.axon_site/_ro/trn_rl_repo"):
    if os.path.isdir(_p) and _p not in sys.path:
        sys.path.append(_p)

import numpy as np
import ml_dtypes

import concourse.bacc as bacc
import concourse.mybir as mybir
from concourse.tile import TileContext
from concourse.bass_utils import run_bass_kernel_spmd

P = 128
NCORES = 8
B_FULL, K_DIM, N_FULL = 8192, 4096, 4096
M_CORE = B_FULL // NCORES  # 1024
KC = K_DIM // 256          # 16 pair-blocks in w layout
MT = M_CORE // P           # 8
NP = N_FULL // 512         # 8 moving panels
DR = mybir.MatmulPerfMode.DoubleRow
F8 = ml_dtypes.float8_e4m3

VARIANT = os.environ.get("BL3_VARIANT", "fp8dr8q")

_NC_CACHE = {}


def _cfg(variant):
    """-> (kc_hi, krc_lo, kt16): fp8-hi blocks, fp8-lo blocks, fp16 k-tiles."""
    if variant == "fp8dr9":
        return 16, 9, 0
    if variant == "fp8dr8":
        return 16, 8, 0
    if variant in ("hyb16", "hyb16a"):
        return 8, 0, 16
    if variant == "fp8dr8q":
        return 16, 8, 0
    raise ValueError(variant)


def _build(variant, repeat=1):
    import contextlib

    kc_hi, krc, kt16 = _cfg(variant)
    dt = mybir.dt
    nc = bacc.Bacc("TRN2")
    wb_d = nc.dram_tensor("wb", [P, KC, 2, N_FULL], dt.float8e4, kind="ExternalInput")
    xh_d = nc.dram_tensor("xh", [P, kc_hi, 2, M_CORE], dt.float8e4,
                          kind="ExternalInput")
    xl_d = (nc.dram_tensor("xl", [P, krc, 2, M_CORE], dt.float8e4,
                           kind="ExternalInput") if krc else None)
    x16_d = (nc.dram_tensor("x16", [P, kt16, M_CORE], dt.float16,
                            kind="ExternalInput") if kt16 else None)
    out_d = nc.dram_tensor("out", [M_CORE, N_FULL], dt.float32,
                           kind="ExternalOutput")

    with TileContext(nc) as tc:
        with (
            tc.tile_pool(name="wpool", bufs=1) as wpool,
            tc.tile_pool(name="xpool", bufs=1) as xpool,
            tc.tile_pool(name="opool", bufs=4) as opool,
            tc.tile_pool(name="pspool", bufs=8, space="PSUM") as pspool,
        ):
            ws = wpool.tile([P, KC, 2, N_FULL], dt.float8e4, name="ws")
            xht = xpool.tile([P, kc_hi, 2, M_CORE], dt.float8e4, name="xht")
            xlt = (xpool.tile([P, krc, 2, M_CORE], dt.float8e4, name="xlt")
                   if krc else None)
            x16t = (xpool.tile([P, kt16, M_CORE], dt.float16, name="x16t")
                    if kt16 else None)
            if variant == "fp8dr8q":
                # hi-phase inputs first (consumption order), lo residuals after
                for kc in range(KC):
                    nc.sync.dma_start(out=xht[:, kc], in_=xh_d[:, kc])
                    nc.sync.dma_start(out=ws[:, kc], in_=wb_d[:, kc])
                for kc in range(krc):
                    nc.sync.dma_start(out=xlt[:, kc], in_=xl_d[:, kc])
            else:
                for kc in range(KC):
                    if kc < kc_hi:
                        nc.sync.dma_start(out=xht[:, kc], in_=xh_d[:, kc])
                    if krc and kc < krc:
                        nc.sync.dma_start(out=xlt[:, kc], in_=xl_d[:, kc])
                    if kt16 and kc < kt16:
                        nc.sync.dma_start(out=x16t[:, kc], in_=x16_d[:, kc])
                    nc.sync.dma_start(out=ws[:, kc], in_=wb_d[:, kc])

            loop_cm = (
                tc.For_i(
                    0, repeat, 1,
                    hint_engines=(mybir.EngineType.PE, mybir.EngineType.SP,
                                  mybir.EngineType.DVE),
                    name="rep",
                ) if repeat > 1 else contextlib.nullcontext()
            )
            with loop_cm:
                for mt in range(MT):
                    pss = [pspool.tile([P, 512], dt.float32, name="ps")
                           for _ in range(NP)]
                    groups = ([("hi", kc) for kc in range(kc_hi)]
                              + [("lo", kc) for kc in range(krc)]
                              + [("f16", kt) for kt in range(kt16)])
                    # ping-pong phase order: halves PE mode switches
                    if variant == "hyb16a" and mt % 2 == 1:
                        groups = groups[::-1]
                    n_groups = len(groups)
                    for g, (kind, idx) in enumerate(groups):
                        for np_ in range(NP):
                            if kind == "hi":
                                nc.tensor.matmul(
                                    pss[np_][:],
                                    lhsT=xht[:, idx, :, mt * P:(mt + 1) * P],
                                    rhs=ws[:, idx, :, np_ * 512:(np_ + 1) * 512],
                                    start=(g == 0), stop=(g == n_groups - 1),
                                    perf_mode=DR, skip_group_check=True,
                                )
                            elif kind == "lo":
                                nc.tensor.matmul(
                                    pss[np_][:],
                                    lhsT=xlt[:, idx, :, mt * P:(mt + 1) * P],
                                    rhs=ws[:, idx, :, np_ * 512:(np_ + 1) * 512],
                                    start=(g == 0), stop=(g == n_groups - 1),
                                    perf_mode=DR, skip_group_check=True,
                                )
                            else:
                                kcw, iw = kc_hi + idx // 2, idx % 2
                                nc.tensor.matmul(
                                    pss[np_][:],
                                    lhsT=x16t[:, idx, mt * P:(mt + 1) * P],
                                    rhs=ws[:, kcw, iw, np_ * 512:(np_ + 1) * 512],
                                    start=(g == 0), stop=(g == n_groups - 1),
                                    skip_group_check=True,
                                )
                    out_eng = nc.scalar if variant == "fp8dr8q" else nc.sync
                    for np_ in range(NP):
                        ot = opool.tile([P, 512], dt.float32, name="ot")
                        nc.vector.tensor_copy(ot[:], pss[np_][:])
                        out_eng.dma_start(
                            out=out_d[mt * P:(mt + 1) * P,
                                      np_ * 512:(np_ + 1) * 512],
                            in_=ot[:],
                        )
    nc.compile()
    return nc


def get_nc(variant=None, repeat=1):
    variant = variant or VARIANT
    key = (variant, repeat)
    if key not in _NC_CACHE:
        _NC_CACHE[key] = _build(variant, repeat)
    return _NC_CACHE[key]


def _pack_k(a, nblk):
    """[nblk*256, cols] -> [128, nblk, 2, cols], k = kc*256 + i*128 + p."""
    n = a.shape[1]
    return np.ascontiguousarray(a.reshape(nblk, 2, P, n).transpose(2, 0, 1, 3))


def prep_in_maps(x, w, variant=None):
    variant = variant or VARIANT
    kc_hi, krc, kt16 = _cfg(variant)
    x = np.asarray(x, dtype=np.float32)
    w = np.asarray(w, dtype=np.float32)
    s8 = np.where(w >= 0, np.float32(1.0), np.float32(-1.0)).astype(F8)
    wb = _pack_k(s8, KC)

    xt = np.ascontiguousarray(x.T)  # [K, B] f32
    xh = xt[:kc_hi * 256].astype(F8)
    if krc:
        r = (xt[:krc * 256] - xh[:krc * 256].astype(np.float32)).astype(F8)
    if kt16:
        # fp16 tiles cover k in [kc_hi*256, 4096); tile kt <-> 128 k rows
        x16 = xt[kc_hi * 256:].astype(np.float16)  # [kt16*128, B]

    in_maps = []
    for c in range(NCORES):
        sl = slice(c * M_CORE, (c + 1) * M_CORE)
        m = {"wb": wb, "xh": _pack_k(np.ascontiguousarray(xh[:, sl]), kc_hi)}
        if krc:
            m["xl"] = _pack_k(np.ascontiguousarray(r[:, sl]), krc)
        if kt16:
            m["x16"] = np.ascontiguousarray(
                x16[:, sl].reshape(kt16, P, M_CORE).transpose(1, 0, 2))
        in_maps.append(m)
    return in_maps


def gather_out(results, variant=None):
    return np.concatenate(
        [np.asarray(results[c]["out"]) for c in range(NCORES)], axis=0)


def kernel(x, w):
    """Full inputs in, full output out.  x [8192,4096] f32, w [4096,4096] f32."""
    assert x.shape == (B_FULL, K_DIM) and w.shape == (K_DIM, N_FULL)
    nc = get_nc()
    in_maps = prep_in_maps(x, w)
    res = run_bass_kernel_spmd(nc, in_maps, core_ids=list(range(NCORES)))
    out = gather_out(res.results)
    return np.ascontiguousarray(out, dtype=np.float32)



# revision 11
# speedup vs baseline: 6.4160x; 6.4160x over previous
"""Trainium2 Bass kernel v3 for nn_BinLinear: out = x @ where(w >= 0, 1, -1).

Variants (BL3_VARIANT env or get_nc(variant)):
  fp8dr9 - fp8 DoubleRow hi over all K + fp8 lo over 2304 k.  err ~1.756e-2
  fp8dr8 - same, lo over 2048 k.                              err ~1.876e-2
  hyb16  - fp8 DR hi-only over k<2048, fp16 normal matmul
           (mixed dtype vs resident fp8 w) over k>=2048.      err ~1.877e-2

All: x row-sharded 8 cores (1024 rows), w resident in SBUF as fp8 e4m3
(+-1 exact), x-stationary (8 moving w panels per LDWEIGHTS), out natural
[1024, 4096] fp32/core, host concat.  DoubleRow pair k-map:
k = kc*256 + i*128 + p.
"""

import os
import sys

for _p in ("/opt/trn_rl_repo", "/root/.axon_site/_ro/trn_rl_repo"):
    if os.path.isdir(_p) and _p not in sys.path:
        sys.path.append(_p)

import numpy as np
import ml_dtypes

import concourse.bacc as bacc
import concourse.mybir as mybir
from concourse.tile import TileContext
from concourse.bass_utils import run_bass_kernel_spmd

P = 128
NCORES = 8
B_FULL, K_DIM, N_FULL = 8192, 4096, 4096
M_CORE = B_FULL // NCORES  # 1024
KC = K_DIM // 256          # 16 pair-blocks in w layout
MT = M_CORE // P           # 8
NP = N_FULL // 512         # 8 moving panels
DR = mybir.MatmulPerfMode.DoubleRow
F8 = ml_dtypes.float8_e4m3

VARIANT = os.environ.get("BL3_VARIANT", "fp8dr8q")

_NC_CACHE = {}


def _cfg(variant):
    """-> (kc_hi, krc_lo, kt16): fp8-hi blocks, fp8-lo blocks, fp16 k-tiles."""
    if variant == "fp8dr9":
        return 16, 9, 0
    if variant == "fp8dr8":
        return 16, 8, 0
    if variant in ("hyb16", "hyb16a"):
        return 8, 0, 16
    if variant == "fp8dr8q":
        return 16, 8, 0
    if variant == "e3dr":
        return 16, 0, 0
    if variant in ("peonly", "drainonly"):
        return 16, 8, 0
    raise ValueError(variant)


def _f8(variant):
    """(mybir fp8 dtype, numpy fp8 dtype) for this variant."""
    if variant.startswith("e3"):
        return mybir.dt.float8e3, ml_dtypes.float8_e3m4
    return mybir.dt.float8e4, F8


def _build(variant, repeat=1):
    import contextlib

    kc_hi, krc, kt16 = _cfg(variant)
    f8m, _ = _f8(variant)
    dt = mybir.dt
    nc = bacc.Bacc("TRN2")
    wb_d = nc.dram_tensor("wb", [P, KC, 2, N_FULL], f8m, kind="ExternalInput")
    xh_d = nc.dram_tensor("xh", [P, kc_hi, 2, M_CORE], f8m,
                          kind="ExternalInput")
    xl_d = (nc.dram_tensor("xl", [P, krc, 2, M_CORE], f8m,
                           kind="ExternalInput") if krc else None)
    x16_d = (nc.dram_tensor("x16", [P, kt16, M_CORE], dt.float16,
                            kind="ExternalInput") if kt16 else None)
    out_d = nc.dram_tensor("out", [M_CORE, N_FULL], dt.float32,
                           kind="ExternalOutput")

    with TileContext(nc) as tc:
        with (
            tc.tile_pool(name="wpool", bufs=1) as wpool,
            tc.tile_pool(name="xpool", bufs=1) as xpool,
            tc.tile_pool(name="opool", bufs=4) as opool,
            tc.tile_pool(name="pspool", bufs=8, space="PSUM") as pspool,
        ):
            ws = wpool.tile([P, KC, 2, N_FULL], f8m, name="ws")
            xht = xpool.tile([P, kc_hi, 2, M_CORE], f8m, name="xht")
            xlt = (xpool.tile([P, krc, 2, M_CORE], f8m, name="xlt")
                   if krc else None)
            x16t = (xpool.tile([P, kt16, M_CORE], dt.float16, name="x16t")
                    if kt16 else None)
            if variant in ("fp8dr8q", "e3dr"):
                # hi-phase inputs first (consumption order), lo residuals after
                for kc in range(KC):
                    nc.sync.dma_start(out=xht[:, kc], in_=xh_d[:, kc])
                    nc.sync.dma_start(out=ws[:, kc], in_=wb_d[:, kc])
                for kc in range(krc):
                    nc.sync.dma_start(out=xlt[:, kc], in_=xl_d[:, kc])
            else:
                for kc in range(KC):
                    if kc < kc_hi:
                        nc.sync.dma_start(out=xht[:, kc], in_=xh_d[:, kc])
                    if krc and kc < krc:
                        nc.sync.dma_start(out=xlt[:, kc], in_=xl_d[:, kc])
                    if kt16 and kc < kt16:
                        nc.sync.dma_start(out=x16t[:, kc], in_=x16_d[:, kc])
                    nc.sync.dma_start(out=ws[:, kc], in_=wb_d[:, kc])

            loop_cm = (
                tc.For_i(
                    0, repeat, 1,
                    hint_engines=(mybir.EngineType.PE, mybir.EngineType.SP,
                                  mybir.EngineType.DVE),
                    name="rep",
                ) if repeat > 1 else contextlib.nullcontext()
            )
            if variant in ("peonly", "drainonly"):
                # timing-only microbenchmarks: wrong output, measures engine
                # rates in isolation.  peonly: pure PE matmul stream with one
                # drain at end.  drainonly: copies+DMA only, no matmuls.
                with loop_cm:
                    pss = [pspool.tile([P, 512], dt.float32, name="ps")
                           for _ in range(NP)]
                    n_groups = (kc_hi + krc) * MT
                    if variant == "peonly":
                        for g in range(n_groups):
                            kc = g % (kc_hi + krc)
                            mt = g // (kc_hi + krc)
                            src = xht if kc < kc_hi else xlt
                            idx = kc if kc < kc_hi else kc - kc_hi
                            for np_ in range(NP):
                                nc.tensor.matmul(
                                    pss[np_][:],
                                    lhsT=src[:, idx, :, mt * P:(mt + 1) * P],
                                    rhs=ws[:, idx % KC, :,
                                           np_ * 512:(np_ + 1) * 512],
                                    start=(g == 0), stop=(g == n_groups - 1),
                                    perf_mode=DR, skip_group_check=True,
                                )
                    else:
                        for np_ in range(NP):
                            nc.tensor.matmul(
                                pss[np_][:],
                                lhsT=xht[:, 0, :, 0:P],
                                rhs=ws[:, 0, :, np_ * 512:(np_ + 1) * 512],
                                start=True, stop=True,
                                perf_mode=DR, skip_group_check=True,
                            )
                        for mt in range(MT):
                            for np_ in range(NP):
                                ot = opool.tile([P, 512], dt.float32, name="ot")
                                nc.vector.tensor_copy(ot[:], pss[np_][:])
                                nc.scalar.dma_start(
                                    out=out_d[mt * P:(mt + 1) * P,
                                              np_ * 512:(np_ + 1) * 512],
                                    in_=ot[:],
                                )
                    if variant == "peonly":
                        for np_ in range(NP):
                            ot = opool.tile([P, 512], dt.float32, name="ot")
                            nc.vector.tensor_copy(ot[:], pss[np_][:])
                            nc.scalar.dma_start(
                                out=out_d[0:P, np_ * 512:(np_ + 1) * 512],
                                in_=ot[:],
                            )
            else:
              with loop_cm:
                for mt in range(MT):
                    pss = [pspool.tile([P, 512], dt.float32, name="ps")
                           for _ in range(NP)]
                    groups = ([("hi", kc) for kc in range(kc_hi)]
                              + [("lo", kc) for kc in range(krc)]
                              + [("f16", kt) for kt in range(kt16)])
                    # ping-pong phase order: halves PE mode switches
                    if variant == "hyb16a" and mt % 2 == 1:
                        groups = groups[::-1]
                    n_groups = len(groups)
                    for g, (kind, idx) in enumerate(groups):
                        for np_ in range(NP):
                            if kind == "hi":
                                nc.tensor.matmul(
                                    pss[np_][:],
                                    lhsT=xht[:, idx, :, mt * P:(mt + 1) * P],
                                    rhs=ws[:, idx, :, np_ * 512:(np_ + 1) * 512],
                                    start=(g == 0), stop=(g == n_groups - 1),
                                    perf_mode=DR, skip_group_check=True,
                                )
                            elif kind == "lo":
                                nc.tensor.matmul(
                                    pss[np_][:],
                                    lhsT=xlt[:, idx, :, mt * P:(mt + 1) * P],
                                    rhs=ws[:, idx, :, np_ * 512:(np_ + 1) * 512],
                                    start=(g == 0), stop=(g == n_groups - 1),
                                    perf_mode=DR, skip_group_check=True,
                                )
                            else:
                                kcw, iw = kc_hi + idx // 2, idx % 2
                                nc.tensor.matmul(
                                    pss[np_][:],
                                    lhsT=x16t[:, idx, mt * P:(mt + 1) * P],
                                    rhs=ws[:, kcw, iw, np_ * 512:(np_ + 1) * 512],
                                    start=(g == 0), stop=(g == n_groups - 1),
                                    skip_group_check=True,
                                )
                    out_eng = (nc.scalar if variant in ("fp8dr8q", "e3dr")
                               else nc.sync)
                    for np_ in range(NP):
                        ot = opool.tile([P, 512], dt.float32, name="ot")
                        nc.vector.tensor_copy(ot[:], pss[np_][:])
                        out_eng.dma_start(
                            out=out_d[mt * P:(mt + 1) * P,
                                      np_ * 512:(np_ + 1) * 512],
                            in_=ot[:],
                        )
    nc.compile()
    return nc


def get_nc(variant=None, repeat=1):
    variant = variant or VARIANT
    key = (variant, repeat)
    if key not in _NC_CACHE:
        _NC_CACHE[key] = _build(variant, repeat)
    return _NC_CACHE[key]


def _pack_k(a, nblk):
    """[nblk*256, cols] -> [128, nblk, 2, cols], k = kc*256 + i*128 + p."""
    n = a.shape[1]
    return np.ascontiguousarray(a.reshape(nblk, 2, P, n).transpose(2, 0, 1, 3))


def prep_in_maps(x, w, variant=None):
    variant = variant or VARIANT
    kc_hi, krc, kt16 = _cfg(variant)
    _, f8np = _f8(variant)
    x = np.asarray(x, dtype=np.float32)
    w = np.asarray(w, dtype=np.float32)
    s8 = np.where(w >= 0, np.float32(1.0), np.float32(-1.0)).astype(f8np)
    wb = _pack_k(s8, KC)

    xt = np.ascontiguousarray(x.T)  # [K, B] f32
    xh = xt[:kc_hi * 256].astype(f8np)
    if krc:
        r = (xt[:krc * 256] - xh[:krc * 256].astype(np.float32)).astype(F8)
    if kt16:
        # fp16 tiles cover k in [kc_hi*256, 4096); tile kt <-> 128 k rows
        x16 = xt[kc_hi * 256:].astype(np.float16)  # [kt16*128, B]

    in_maps = []
    for c in range(NCORES):
        sl = slice(c * M_CORE, (c + 1) * M_CORE)
        m = {"wb": wb, "xh": _pack_k(np.ascontiguousarray(xh[:, sl]), kc_hi)}
        if krc:
            m["xl"] = _pack_k(np.ascontiguousarray(r[:, sl]), krc)
        if kt16:
            m["x16"] = np.ascontiguousarray(
                x16[:, sl].reshape(kt16, P, M_CORE).transpose(1, 0, 2))
        in_maps.append(m)
    return in_maps


def gather_out(results, variant=None):
    return np.concatenate(
        [np.asarray(results[c]["out"]) for c in range(NCORES)], axis=0)


def kernel(x, w):
    """Full inputs in, full output out.  x [8192,4096] f32, w [4096,4096] f32."""
    assert x.shape == (B_FULL, K_DIM) and w.shape == (K_DIM, N_FULL)
    nc = get_nc()
    in_maps = prep_in_maps(x, w)
    res = run_bass_kernel_spmd(nc, in_maps, core_ids=list(range(NCORES)))
    out = gather_out(res.results)
    return np.ascontiguousarray(out, dtype=np.float32)

